# revision 1
# baseline (speedup 1.0000x reference)
import sys

if "/opt/trn_rl_repo" not in sys.path:
    sys.path.insert(0, "/opt/trn_rl_repo")

import numpy as np

NCORES = 8
B = 65536
NPC = B // NCORES  # 8192 images per core
G = 8              # image-tiles (of 128) per group
NGROUPS = NPC // (128 * G)
MAGIC = 12582912.0  # 1.5 * 2**23: (v+M)-M == round-to-nearest-even, |v| < 2**22
AF = 128.0 / 127.5

_cache = {}


def _build(wq9, ndve=5):
    """wq9: tuple of 9 floats, quantized conv taps in {0,+-0.5}, row-major.
    ndve: how many of the non-center taps run on DVE (rest on Pool)."""
    from contextlib import ExitStack

    import concourse.tile as tile
    from concourse import bacc, mybir

    f32 = mybir.dt.float32
    f16 = mybir.dt.float16
    Alu = mybir.AluOpType
    Act = mybir.ActivationFunctionType

    nc = bacc.Bacc("TRN2", target_bir_lowering=False, debug=False,
                   num_devices=NCORES)
    x = nc.dram_tensor("x", [NPC, 576], f32, kind="ExternalInput").ap()
    wfc = nc.dram_tensor("wfc", [256, 10], f16, kind="ExternalInput").ap()
    out = nc.dram_tensor("out", [10, NPC], f32, kind="ExternalOutput").ap()

    with tile.TileContext(nc) as tc, ExitStack() as ctx:
        consts = ctx.enter_context(tc.tile_pool(name="consts", bufs=1))
        w1 = consts.tile([128, 10], f16)
        w2 = consts.tile([128, 10], f16)
        nc.sync.dma_start(w1[:], wfc[0:128, :])
        nc.sync.dma_start(w2[:], wfc[128:256, :])

        xpool = ctx.enter_context(tc.tile_pool(name="xp", bufs=2))
        hpool = ctx.enter_context(tc.tile_pool(name="hp", bufs=2))
        yapool = ctx.enter_context(tc.tile_pool(name="yap", bufs=2))
        ybpool = ctx.enter_context(tc.tile_pool(name="ybp", bufs=2))
        ppool = ctx.enter_context(tc.tile_pool(name="pp", bufs=2))
        apool = ctx.enter_context(tc.tile_pool(name="ap", bufs=2))
        tpool = ctx.enter_context(tc.tile_pool(name="tp", bufs=4))
        spool = ctx.enter_context(tc.tile_pool(name="sp", bufs=2))
        po = ctx.enter_context(tc.tile_pool(name="po", bufs=2, space="PSUM"))

        xv_dram = x.rearrange("(g a p) f -> g p a f", p=128, a=G)

        # taps scaled x2 so they land in {0,+-1}: pure add/subtract on A/2
        cen = 2.0 * wq9[4]
        taps = [(dr, dc, 2.0 * wq9[(dr + 1) * 3 + (dc + 1)])
                for dr in (-1, 0, 1) for dc in (-1, 0, 1)
                if not (dr == 0 and dc == 0)
                and wq9[(dr + 1) * 3 + (dc + 1)] != 0.0]
        dve_taps = taps[:ndve]
        pool_taps = taps[ndve:]

        R = G * 24
        for g in range(NGROUPS):
            xt = xpool.tile([128, G * 576], f32)
            nc.sync.dma_start(xt[:].rearrange("p (a f) -> p a f", a=G),
                              xv_dram[g])
            # quantize: A = clamp(round(x*AF - 128), -127, 127); xh = A/2 fp16
            nc.scalar.activation(xt[:], xt[:], Act.Copy,
                                 bias=MAGIC - 128.0, scale=AF)
            nc.vector.tensor_scalar(xt[:], xt[:], MAGIC, -127.0,
                                    Alu.subtract, Alu.max)
            xh = hpool.tile([128, G * 576], f16)
            nc.gpsimd.tensor_scalar(xh[:], xt[:], 127.0, 0.5,
                                    Alu.min, Alu.mult)

            # 3x3 SAME conv (x128 domain) as shifted +-xh adds, split across
            # two accumulators so DVE and Pool run independent chains.
            ya = yapool.tile([128, G * 576], f16)
            yb = ybpool.tile([128, G * 576], f16)
            nc.scalar.activation(ya[:], xh[:], Act.Copy, bias=0.0, scale=cen)
            nc.gpsimd.tensor_scalar_mul(yb[:], xh[:], 0.0)

            xr = xh[:].rearrange("p (r w) -> p r w", w=24)
            xa = xh[:].rearrange("p (a f) -> p a f", a=G)
            for eng, yt, tlist in ((nc.vector, ya, dve_taps),
                                   (nc.gpsimd, yb, pool_taps)):
                yr = yt[:].rearrange("p (r w) -> p r w", w=24)
                yv = yt[:].rearrange("p (a f) -> p a f", a=G)
                for dr, dc, s in tlist:
                    op = Alu.add if s > 0 else Alu.subtract
                    cop = Alu.subtract if s > 0 else Alu.add
                    co0, co1 = max(0, -dc), 24 - max(0, dc)
                    if dr == 0:
                        eng.tensor_tensor(yr[:, :, co0:co1],
                                          yr[:, :, co0:co1],
                                          xr[:, :, co0 + dc:co1 + dc], op)
                        continue
                    r0, r1 = max(0, -dr), R - max(0, dr)
                    eng.tensor_tensor(
                        yr[:, r0:r1, co0:co1], yr[:, r0:r1, co0:co1],
                        xr[:, r0 + dr:r1 + dr, co0 + dc:co1 + dc], op)
                    # cancel cross-image leakage on the G-1 boundary rows
                    if dr == 1:
                        ysl = yv[:, 0:G - 1, 23 * 24 + co0:23 * 24 + co1]
                        xsl = xa[:, 1:G, co0 + dc:co1 + dc]
                    else:
                        ysl = yv[:, 1:G, co0:co1]
                        xsl = xa[:, 0:G - 1,
                                 23 * 24 + co0 + dc:23 * 24 + co1 + dc]
                    eng.tensor_tensor(ysl, ysl, xsl, cop)

            nc.vector.tensor_tensor(ya[:], ya[:], yb[:], Alu.add)

            # maxpool 2x2 -> 12x12 interior (pad ring pools to zero, dropped)
            p1 = ppool.tile([128, G * 288], f16)
            yv4 = ya[:].rearrange("p (r t w) -> p r t w", t=2, w=24)
            p1r = p1[:].rearrange("p (r w) -> p r w", w=24)
            nc.vector.tensor_tensor(p1r, yv4[:, :, 0, :], yv4[:, :, 1, :],
                                    Alu.max)
            act = apool.tile([128, G * 144], f16)
            p1v4 = p1[:].rearrange("p (r w t) -> p r w t", w=12, t=2)
            actr = act[:].rearrange("p (r w) -> p r w", w=12)
            nc.vector.tensor_tensor(actr, p1v4[:, :, :, 0], p1v4[:, :, :, 1],
                                    Alu.max)
            # relu + clip 127 + round (fp16 magic 1536 = 1.5*2**10)
            nc.vector.tensor_scalar(act[:], act[:], 0.0, 127.0,
                                    Alu.max, Alu.min)
            nc.vector.tensor_scalar(act[:], act[:], 1536.0, 1536.0,
                                    Alu.add, Alu.subtract)

            # FC: out^T[o, b] = sum_k W[k, o] actT[k, b], K=144 as two
            # 128-partition matmuls: actT of feats 0:128 vs W_A, and of
            # feats 16:144 vs W_B (zeros except rows 112:128 = feats 128:144)
            for h in range(2):
                aT1 = tpool.tile([128, 512], f16)
                aT2 = tpool.tile([128, 512], f16)
                for j in range(4):
                    a = h * 4 + j
                    nc.sync.dma_start_transpose(
                        aT1[:, j * 128:(j + 1) * 128],
                        act[:, a * 144:a * 144 + 128])
                    nc.sync.dma_start_transpose(
                        aT2[:, j * 128:(j + 1) * 128],
                        act[:, a * 144 + 16:a * 144 + 144])
                pOT = po.tile([10, 512], f32)
                nc.tensor.matmul(pOT[:], w1[:], aT1[:], start=True, stop=False)
                nc.tensor.matmul(pOT[:], w2[:], aT2[:], start=False, stop=True)
                soT = spool.tile([10, 512], f32)
                nc.scalar.copy(soT[:], pOT[:])
                nc.sync.dma_start(
                    out[:, g * 1024 + h * 512:g * 1024 + (h + 1) * 512],
                    soT[:])

    nc.compile()
    return nc


def _prep(conv_w, fc_w):
    # replicate reference weight quantization exactly (all steps exact in f32)
    cw = np.asarray(conv_w, np.float32).reshape(3, 3)
    wq = (np.round(np.clip(cw, -0.5, 0.5) * 2.0) / 2.0).astype(np.float32)
    fw = np.asarray(fc_w, np.float32)
    wfq = (np.round(np.clip(fw, -0.5, 0.5) * 2.0) / 2.0 / 8.0).astype(np.float32)
    # FC sees act128/128; fold the /128 into W (values k/2048, exact fp16).
    # Rows 0:128 = feats 0:128 (W_A); rows 240:256 = feats 128:144 placed at
    # partition 112+ of W_B to match the feats-16:144 transposed tile.
    Wdev = np.zeros((256, 10), np.float32)
    for i in range(12):
        for j in range(12):
            k = i * 12 + j
            r = k if k < 128 else k + 112
            Wdev[r, :] = wfq[:, (i + 1) * 14 + (j + 1)] / 128.0
    return tuple(float(v) for v in wq.flatten()), Wdev.astype(np.float16)


def _get_program(wq9, ndve=5):
    key = (wq9, ndve)
    nc = _cache.get(key)
    if nc is None:
        nc = _build(wq9, ndve)
        _cache[key] = nc
    return nc


def _make_in_maps(x2d, Wdev):
    return [{"x": np.ascontiguousarray(x2d[c * NPC:(c + 1) * NPC]),
             "wfc": Wdev} for c in range(NCORES)]


def run(x, conv_w, fc_w, trace=False, **kw):
    from concourse.bass_utils import run_bass_kernel_spmd

    x2d = np.ascontiguousarray(
        np.asarray(x, np.float32).reshape(B, 576))
    wq9, Wdev = _prep(conv_w, fc_w)
    nc = _get_program(wq9)
    res = run_bass_kernel_spmd(nc, _make_in_maps(x2d, Wdev),
                               core_ids=list(range(NCORES)),
                               trace=trace, **kw)
    out = np.concatenate([np.asarray(r["out"]).T for r in res.results], axis=0)
    return np.ascontiguousarray(out.astype(np.float32)), res


def kernel(x, conv_w, fc_w):
    out, _ = run(x, conv_w, fc_w, trace=False)
    return out



# revision 3
# speedup vs baseline: 5.8348x; 5.8348x over previous
import sys

if "/opt/trn_rl_repo" not in sys.path:
    sys.path.insert(0, "/opt/trn_rl_repo")

import numpy as np

NCORES = 8
B = 65536
NPC = B // NCORES  # 8192 images per core
G = 8              # image-tiles (of 128) per group
NGROUPS = NPC // (128 * G)
AF = 128.0 / 127.5

_cache = {}


def _build(wq9):
    """wq9: tuple of 9 floats, quantized conv taps in {0,+-0.5}, row-major.

    Engine plan (GpSimd deliberately unused -- it stalls DVE via the shared
    SBUF port): Scalar does the f32->f16 quant affine + post-pool unshift +
    PSUM->SBUF bounces; DVE does clamps, conv shift-adds, pools, rounding;
    TensorE does act transposes + the FC matmuls."""
    from contextlib import ExitStack

    import concourse.tile as tile
    from concourse import bacc, mybir

    f32 = mybir.dt.float32
    f16 = mybir.dt.float16
    Alu = mybir.AluOpType
    Act = mybir.ActivationFunctionType

    nc = bacc.Bacc("TRN2", target_bir_lowering=False, debug=False,
                   num_devices=NCORES)
    x = nc.dram_tensor("x", [NPC, 576], f32, kind="ExternalInput").ap()
    wfc = nc.dram_tensor("wfc", [256, 10], f16, kind="ExternalInput").ap()
    ident = nc.dram_tensor("ident", [128, 128], f16, kind="ExternalInput").ap()
    out = nc.dram_tensor("out", [10, NPC], f32, kind="ExternalOutput").ap()

    with tile.TileContext(nc) as tc, ExitStack() as ctx:
        consts = ctx.enter_context(tc.tile_pool(name="consts", bufs=1))
        w1 = consts.tile([128, 10], f16)
        w2 = consts.tile([128, 10], f16)
        idt = consts.tile([128, 128], f16)
        nc.sync.dma_start(w1[:], wfc[0:128, :])
        nc.sync.dma_start(w2[:], wfc[128:256, :])
        nc.sync.dma_start(idt[:], ident[:, :])

        xpool = ctx.enter_context(tc.tile_pool(name="xp", bufs=2))
        hpool = ctx.enter_context(tc.tile_pool(name="hp", bufs=2))
        ypool = ctx.enter_context(tc.tile_pool(name="yp", bufs=2))
        ppool = ctx.enter_context(tc.tile_pool(name="pp", bufs=2))
        apool = ctx.enter_context(tc.tile_pool(name="ap", bufs=2))
        a2pool = ctx.enter_context(tc.tile_pool(name="a2p", bufs=2))
        tpool = ctx.enter_context(tc.tile_pool(name="tp", bufs=4))
        spool = ctx.enter_context(tc.tile_pool(name="sp", bufs=2))
        pt = ctx.enter_context(tc.tile_pool(name="pt", bufs=2, space="PSUM"))
        po = ctx.enter_context(tc.tile_pool(name="po", bufs=2, space="PSUM"))

        xv_dram = x.rearrange("(g a p) f -> g p a f", p=128, a=G)

        # taps scaled x2 so they land in {0,+-1}: pure add/subtract on A/2
        cen = 2.0 * wq9[4]
        taps = [(dr, dc, 2.0 * wq9[(dr + 1) * 3 + (dc + 1)])
                for dr in (-1, 0, 1) for dc in (-1, 0, 1)
                if not (dr == 0 and dc == 0)
                and wq9[(dr + 1) * 3 + (dc + 1)] != 0.0]

        R = G * 24
        FD = G * 576

        def load(g):
            xt = xpool.tile([128, FD], f32)
            nc.sync.dma_start(xt[:].rearrange("p (a f) -> p a f", a=G),
                              xv_dram[g])
            return xt

        def quant_scalar(xt):
            # t = AF/2*x + 640; f16 store rounds to the 0.5 grid (ULP=0.5
            # over [512,1024)), reproducing round(AF*x-128)/2 + 704 exactly.
            t = hpool.tile([128, FD], f16)
            nc.scalar.activation(t[:], xt[:], Act.Copy, bias=640.0,
                                 scale=AF / 2.0)
            return t

        xt_cur = load(0)
        t_cur = quant_scalar(xt_cur)

        for g in range(NGROUPS):
            xt_nxt = load(g + 1) if g + 1 < NGROUPS else None

            # clamp to +-63.5: xh = A/2 where A = clamp(round(AF*x-128),+-127)
            nc.vector.tensor_scalar(t_cur[:], t_cur[:], 704.0, -63.5,
                                    Alu.subtract, Alu.max)
            nc.vector.tensor_scalar_min(t_cur[:], t_cur[:], 63.5)
            xh = t_cur

            ya = ypool.tile([128, FD], f16)
            nc.vector.tensor_scalar_mul(ya[:], xh[:], cen)

            # scalar engine prepares next group's t while DVE runs the conv
            t_nxt = quant_scalar(xt_nxt) if xt_nxt is not None else None

            # 3x3 SAME conv (x128 domain) as shifted +-xh adds on DVE
            xr = xh[:].rearrange("p (r w) -> p r w", w=24)
            xa = xh[:].rearrange("p (a f) -> p a f", a=G)
            yr = ya[:].rearrange("p (r w) -> p r w", w=24)
            yv = ya[:].rearrange("p (a f) -> p a f", a=G)
            for dr, dc, s in taps:
                op = Alu.add if s > 0 else Alu.subtract
                cop = Alu.subtract if s > 0 else Alu.add
                co0, co1 = max(0, -dc), 24 - max(0, dc)
                if dr == 0:
                    nc.vector.tensor_tensor(yr[:, :, co0:co1],
                                            yr[:, :, co0:co1],
                                            xr[:, :, co0 + dc:co1 + dc], op)
                    continue
                r0, r1 = max(0, -dr), R - max(0, dr)
                nc.vector.tensor_tensor(
                    yr[:, r0:r1, co0:co1], yr[:, r0:r1, co0:co1],
                    xr[:, r0 + dr:r1 + dr, co0 + dc:co1 + dc], op)
                # cancel cross-image leakage on the G-1 boundary rows
                if dr == 1:
                    ysl = yv[:, 0:G - 1, 23 * 24 + co0:23 * 24 + co1]
                    xsl = xa[:, 1:G, co0 + dc:co1 + dc]
                else:
                    ysl = yv[:, 1:G, co0:co1]
                    xsl = xa[:, 0:G - 1,
                             23 * 24 + co0 + dc:23 * 24 + co1 + dc]
                nc.vector.tensor_tensor(ysl, ysl, xsl, cop)

            # maxpool 2x2 -> 12x12 interior (pad ring pools to zero, dropped)
            p1 = ppool.tile([128, G * 288], f16)
            yv4 = ya[:].rearrange("p (r t w) -> p r t w", t=2, w=24)
            p1r = p1[:].rearrange("p (r w) -> p r w", w=24)
            nc.vector.tensor_tensor(p1r, yv4[:, :, 0, :], yv4[:, :, 1, :],
                                    Alu.max)
            # fold relu into the column-pair max: act = max(max(even,0), odd)
            act = apool.tile([128, G * 144], f16)
            p1v4 = p1[:].rearrange("p (r w t) -> p r w t", w=12, t=2)
            actr = act[:].rearrange("p (r w) -> p r w", w=12)
            nc.vector.scalar_tensor_tensor(actr, p1v4[:, :, :, 0], 0.0,
                                           p1v4[:, :, :, 1], Alu.max, Alu.max)
            # clip 127 + round to int via f16 magic (+1536: ULP=1 there)
            nc.vector.tensor_scalar(act[:], act[:], 127.0, 1536.0,
                                    Alu.min, Alu.add)
            # unshift on scalar: act2 holds exact ints in [0,127]
            act2 = a2pool.tile([128, G * 144], f16)
            nc.scalar.activation(act2[:], act[:], Act.Copy, bias=-1536.0)

            # FC via TensorE: transpose act2 into PSUM, bounce to SBUF,
            # then out^T[o,b] = sum_k W[k,o] actT[k,b] (K=144 as two
            # 128-partition matmuls; W_B rows 112:128 = feats 128:144)
            for h in range(2):
                pA = pt.tile([128, 512], f16)
                pB = pt.tile([128, 512], f16)
                for j in range(4):
                    a = h * 4 + j
                    nc.tensor.transpose(pA[:, j * 128:(j + 1) * 128],
                                        act2[:, a * 144:a * 144 + 128],
                                        idt[:])
                    nc.tensor.transpose(pB[:, j * 128:(j + 1) * 128],
                                        act2[:, a * 144 + 16:a * 144 + 144],
                                        idt[:])
                aT1 = tpool.tile([128, 512], f16)
                aT2 = tpool.tile([128, 512], f16)
                nc.scalar.copy(aT1[:], pA[:])
                nc.scalar.copy(aT2[:], pB[:])
                pOT = po.tile([10, 512], f32)
                nc.tensor.matmul(pOT[:], w1[:], aT1[:], start=True, stop=False)
                nc.tensor.matmul(pOT[:], w2[:], aT2[:], start=False, stop=True)
                soT = spool.tile([10, 512], f32)
                nc.scalar.copy(soT[:], pOT[:])
                nc.sync.dma_start(
                    out[:, g * 1024 + h * 512:g * 1024 + (h + 1) * 512],
                    soT[:])

            xt_cur, t_cur = xt_nxt, t_nxt

    nc.compile()
    return nc


def _prep(conv_w, fc_w):
    # replicate reference weight quantization exactly (all steps exact in f32)
    cw = np.asarray(conv_w, np.float32).reshape(3, 3)
    wq = (np.round(np.clip(cw, -0.5, 0.5) * 2.0) / 2.0).astype(np.float32)
    fw = np.asarray(fc_w, np.float32)
    wfq = (np.round(np.clip(fw, -0.5, 0.5) * 2.0) / 2.0 / 8.0).astype(np.float32)
    # FC sees act*1 (ints 0..127) vs reference act/128; fold the /128 into W
    # (values k/2048, exact fp16). Rows 0:128 = feats 0:128 (W_A); rows
    # 240:256 = feats 128:144 placed at partition 112+ of W_B to match the
    # feats-16:144 transposed tile.
    Wdev = np.zeros((256, 10), np.float32)
    for i in range(12):
        for j in range(12):
            k = i * 12 + j
            r = k if k < 128 else k + 112
            Wdev[r, :] = wfq[:, (i + 1) * 14 + (j + 1)] / 128.0
    return tuple(float(v) for v in wq.flatten()), Wdev.astype(np.float16)


def _get_program(wq9):
    nc = _cache.get(wq9)
    if nc is None:
        nc = _build(wq9)
        _cache[wq9] = nc
    return nc


_IDENT = np.eye(128, dtype=np.float16)


def _make_in_maps(x2d, Wdev):
    return [{"x": np.ascontiguousarray(x2d[c * NPC:(c + 1) * NPC]),
             "wfc": Wdev, "ident": _IDENT} for c in range(NCORES)]


def run(x, conv_w, fc_w, trace=False, **kw):
    from concourse.bass_utils import run_bass_kernel_spmd

    x2d = np.ascontiguousarray(
        np.asarray(x, np.float32).reshape(B, 576))
    wq9, Wdev = _prep(conv_w, fc_w)
    nc = _get_program(wq9)
    res = run_bass_kernel_spmd(nc, _make_in_maps(x2d, Wdev),
                               core_ids=list(range(NCORES)),
                               trace=trace, **kw)
    out = np.concatenate([np.asarray(r["out"]).T for r in res.results], axis=0)
    return np.ascontiguousarray(out.astype(np.float32)), res


def kernel(x, conv_w, fc_w):
    out, _ = run(x, conv_w, fc_w, trace=False)
    return out
